# revision 25
# baseline (speedup 1.0000x reference)
"""MVS plane-sweep cost-volume kernel for Trainium2 (Bass/Tile), 8 NeuronCores.

v5: batched pipeline.
  - dma_gather calls batched 4 depth-planes at a time (num_idxs=4096,
    512B pair descriptors) -> 131 gathers instead of 522, amortizing the
    ~1us SWDGE fixed cost that made GpSimd the bottleneck.
  - idx/weight side-tables and outputs batched to one HWDGE DMA per
    sc-column ([128,3072] tiles) instead of 522 tiny [128,64] transfers.
  - DVE ops span 4 depth planes per instruction ([128,8192] mul with a
    stride-0 broadcast of the ref stream, segmented reduces); channel
    reduce emits f16 to enable the 2x DVE perf mode.
  Host-side pair-table construction unchanged from v4-NT (512B pair
  descriptors, dual-parity table, solo re-gather stream for bad pairs).
"""
import numpy as np

B, V, C, H, W = 2, 5, 32, 128, 160
D = 48
HW = H * W
NCORES = 8
PAD = W + 1
NZ = HW + W + 1                 # 20641 base cells
NE = NZ + 3                     # src pair-table rows: row r = [Z4(r-2), Z4(r-1)]
NR = HW // 2                    # ref pair-table rows
SC = 2048                       # pixels per depth-plane chunk (1024 pairs)
NSC = HW // SC                  # 10
DG = 4                          # depth planes per gather call
DB = D // DG                    # 12 gather blocks per sc
NSOLO_CH = 44                   # solo chunks of 1024 units (padded, worst ~41k)
SBLK = NSOLO_CH // DG           # 11 solo gather blocks
NSOLO = NSOLO_CH * 1024

_PROGRAM_CACHE = {}


def _fold(proj):
    out = proj[0].copy()
    out[:3, :4] = (proj[1][:3, :3] @ proj[0][:3, :4]).astype(np.float32)
    return out


def _host_fields(features, proj_matrices, depth_values, view_weights):
    ys, xs = np.meshgrid(np.arange(H, dtype=np.float32),
                         np.arange(W, dtype=np.float32), indexing='ij')
    grid = np.stack([xs.ravel(), ys.ravel(), np.ones(HW, dtype=np.float32)], 0)
    cores = []
    for b in range(B):
        ref_p = _fold(proj_matrices[b, 0])
        ref_p_inv = np.linalg.inv(ref_p.astype(np.float64)).astype(np.float32)
        for v in range(1, V):
            proj = (_fold(proj_matrices[b, v]).astype(np.float64)
                    @ ref_p_inv.astype(np.float64)).astype(np.float32)
            rot, trans = proj[:3, :3], proj[:3, 3]
            rot_xyz = rot.astype(np.float32) @ grid
            dep = depth_values[b].astype(np.float32)
            pxyz = (rot_xyz[:, None, :] * dep[None, :, None]
                    + trans[:, None, None]).astype(np.float32)
            px = (pxyz[0] / pxyz[2]).astype(np.float32)
            py = (pxyz[1] / pxyz[2]).astype(np.float32)
            x0 = np.floor(px)
            y0 = np.floor(py)
            wx = px - x0
            wy = py - y0
            vw = view_weights[b, v - 1].reshape(HW)
            x0c = np.clip(x0, -1, W - 1)
            y0c = np.clip(y0, -1, H - 1)
            q = (y0c * W + x0c + PAD).astype(np.int64)
            wt4 = np.empty((4, D, HW), dtype=np.float32)
            corners = [(x0, y0, (1 - wx) * (1 - wy)),
                       (x0 + 1, y0, wx * (1 - wy)),
                       (x0, y0 + 1, (1 - wx) * wy),
                       (x0 + 1, y0 + 1, wx * wy)]
            for k, (xi, yi, wk) in enumerate(corners):
                valid = ((xi >= 0) & (xi <= W - 1) & (yi >= 0) & (yi <= H - 1))
                wt4[k] = (wk * valid).astype(np.float32) * vw[None, :] / np.float32(C)
            cores.append((b, v, q, wt4))
    return cores


def _e_src(b):
    """base b (array, in [-2, NZ]) -> sliding-window pair-table row (slot0=b)."""
    return (b + 2).astype(np.int64)


def _build_tables(src, ref):
    """-> combined pair table [NE + NR, 256] f16; row = 2 x (s,t,c) packs."""
    npos = NE + 2
    bb = np.arange(npos) - 2
    f4 = np.empty((128, npos), dtype=np.float16)
    r4 = np.empty((128, HW), dtype=np.float16)
    for s in (0, 1):
        for t in (0, 1):
            rows = slice(s * 64 + t * 32, s * 64 + t * 32 + 32)
            f4[rows] = src[:, np.clip(bb - PAD + s * W + t, 0, HW - 1)]
            r4[rows] = ref
    tbl = np.empty((NE + NR, 2, 128), dtype=np.float16)
    rr = np.arange(NE)
    tbl[:NE, 0, :] = f4[:, rr].T             # slot0 = base r-2 (pos r)
    tbl[:NE, 1, :] = f4[:, rr + 1].T         # slot1 = base r-1
    tbl[NE:, 0, :] = r4[:, 0::2].T
    tbl[NE:, 1, :] = r4[:, 1::2].T
    return tbl.reshape(NE + NR, 256)


def _wrap16(idx2d):
    *lead, n = idx2d.shape
    w = idx2d.reshape(*lead, n // 16, 16)
    w = np.swapaxes(w, -1, -2)
    return np.tile(w, (*([1] * len(lead)), 8, 1)).astype(np.int16)


def _pack_core_inputs(features, cores):
    in_maps = []
    for (b, v, q, wt4) in cores:
        src = features[b, v].reshape(C, HW).astype(np.float32)
        ref = features[b, 0].reshape(C, HW).astype(np.float32)
        tbl = _build_tables(src, ref)

        qa = q[:, 0::2]
        qb = q[:, 1::2]
        good = (qb == qa + 1)

        # gather idx stream: 4 depth planes per call -> [NSC,128,DB*256] i16
        idx_all = _e_src(qa).reshape(D, NSC, SC // 2)
        idx_blk = (idx_all.reshape(DB, DG, NSC, SC // 2)
                   .transpose(2, 0, 1, 3).reshape(NSC, DB, DG * (SC // 2)))
        idx_pk = (_wrap16(idx_blk)                       # (NSC, DB, 128, 256)
                  .transpose(0, 2, 1, 3).reshape(NSC, 128, DB * 256))

        # ref stream per sc: rr[sc, p, slab, v] = ref4[v%128, pixel]
        # pixel(sc, slab, p, px) = sc*2048 + slab*256 + 2p + px, px = v//128
        r4 = np.tile(ref, (4, 1)).astype(np.float16)          # (128, HW)
        pix = (np.arange(NSC)[:, None, None, None] * 2048
               + np.arange(8)[None, :, None, None] * 256
               + np.arange(128)[None, None, :, None] * 2
               + np.arange(2)[None, None, None, :])           # (NSC,8,128,2)
        rr = r4[:, pix]                                       # (128,NSC,8,128,2)
        rr = rr.transpose(1, 3, 2, 4, 0).reshape(NSC, 128, 8 * 256)

        # weights with bad-odd-pixels zeroed; layout [NSC,128,(d,slab,px,k)]
        wt4z = wt4.copy()
        wt4z[:, :, 1::2] *= good[None, :, :]
        wtt = (wt4z.astype(np.float16)
               .reshape(4, D, NSC, 8, 128, 2)
               .transpose(1, 2, 4, 3, 5, 0)                   # d,sc,p,slab,px,k
               .reshape(D, NSC, 128, 64))
        wtt_pk = wtt.transpose(1, 2, 0, 3).reshape(NSC, 128, D * 64)

        # global solo stream, sorted by base for gather locality
        dd, ii = np.nonzero(~good)
        bq = qb[~good]                                        # bases of bad odd pixels
        order = np.argsort(bq, kind="stable")
        dd, ii, bq = dd[order], ii[order], bq[order]
        nsolo = len(dd)
        assert nsolo <= NSOLO, nsolo
        sidx = np.zeros(NSOLO, dtype=np.int64)
        sidx[:nsolo] = _e_src(bq - 1)
        ws = np.zeros((NSOLO, 4), dtype=np.float16)
        ws[:nsolo] = wt4[:, dd, 2 * ii + 1].T.astype(np.float16)
        # dense host-side ref stream for solo units (both slots = ref4 of B)
        r4f = np.tile(ref, (4, 1)).astype(np.float16)         # (128, HW)
        r1 = np.zeros((NSOLO, 128), dtype=np.float16)
        r1[:nsolo] = r4f[:, 2 * ii + 1].T
        rsolo = (np.concatenate([r1, r1], axis=1)             # (NSOLO, 256)
                 .reshape(SBLK, DG, 8, 128, 256)              # blk,dd,slab,p,v
                 .transpose(0, 3, 1, 2, 4)                    # blk,p,dd,slab,v
                 .reshape(SBLK, 128, DG * 8 * 256))
        sidx_pk = (_wrap16(sidx.reshape(SBLK, DG * 1024))     # (SBLK,128,256)
                   .transpose(1, 0, 2).reshape(128, SBLK * 256))
        # unit u -> chunk u//1024, p=(u%1024)%128, slab=(u%1024)//128, px=1
        wsolo = np.zeros((NSOLO_CH, 128, 8, 2, 4), dtype=np.float16)
        u = np.arange(NSOLO)
        wsolo[u // 1024, (u % 1024) % 128, (u % 1024) // 128, 1, :] = ws
        wsolo_pk = (wsolo.reshape(NSOLO_CH, 128, 64)
                    .transpose(1, 0, 2).reshape(128, NSOLO_CH * 64))

        in_maps.append({
            "tbl": np.ascontiguousarray(tbl),
            "rr": np.ascontiguousarray(rr),
            "idx": np.ascontiguousarray(idx_pk),
            "wtt": np.ascontiguousarray(wtt_pk),
            "sidx": np.ascontiguousarray(sidx_pk),
            "rsolo": np.ascontiguousarray(rsolo),
            "wsolo": np.ascontiguousarray(wsolo_pk),
            "_scatter": (dd, 2 * ii + 1),
        })
    return in_maps


def _build_program(queues=None):
    import concourse.bacc as bacc
    import concourse.tile as tile
    import concourse.mybir as mybir

    nc = bacc.Bacc("TRN2", target_bir_lowering=False, debug=False,
                   num_devices=NCORES, num_swdge_queues=4)
    gather_names = []

    def _gather_q():
        g = len(gather_names)
        return queues[g] if queues is not None and g < len(queues) else 0
    f32 = mybir.dt.float32
    f16 = mybir.dt.float16
    i16 = mybir.dt.int16
    X = mybir.AxisListType.X
    ADD = mybir.AluOpType.add

    tbl_d = nc.dram_tensor("tbl", [NE + NR, 256], f16, kind="ExternalInput")
    rr_d = nc.dram_tensor("rr", [NSC, 128, 2048], f16, kind="ExternalInput")
    idx_d = nc.dram_tensor("idx", [NSC, 128, DB * 256], i16, kind="ExternalInput")
    wtt_d = nc.dram_tensor("wtt", [NSC, 128, D * 64], f16, kind="ExternalInput")
    sidx_d = nc.dram_tensor("sidx", [128, SBLK * 256], i16, kind="ExternalInput")
    rsolo_d = nc.dram_tensor("rsolo", [SBLK, 128, 8192], f16,
                             kind="ExternalInput")
    wsolo_d = nc.dram_tensor("wsolo", [128, NSOLO_CH * 64], f16,
                             kind="ExternalInput")
    outg_d = nc.dram_tensor("outg", [NSC, 128, D * 16], f32,
                            kind="ExternalOutput")
    outs_d = nc.dram_tensor("outs", [128, NSOLO_CH * 16], f32,
                            kind="ExternalOutput")

    with tile.TileContext(nc) as tc:
        def tree_reduce(src, tr, c1, wt_slice, ob_slice):
            """src [128,8192]=(a256,c32) prod -> pairwise-add tree (f16 TT
            2x mode; plain tensor_reduce only runs 1x) -> c1 [128,256],
            then corner-weight mul + 4-way reduce into ob_slice."""
            sv = src[:, :].rearrange("p (a c) -> p a c", c=32)
            a1 = tr[:, 0:4096].rearrange("p (a c) -> p a c", c=16)
            nc.vector.tensor_add(a1, sv[:, :, 0:16], sv[:, :, 16:32])
            lvl = [(0, 16), (4096, 8), (6144, 4), (7168, 2)]
            for (o0, w0), (o1, w1) in zip(lvl, lvl[1:]):
                s = tr[:, o0:o0 + 256 * w0].rearrange("p (a c) -> p a c", c=w0)
                d = tr[:, o1:o1 + 256 * w1].rearrange("p (a c) -> p a c", c=w1)
                nc.vector.tensor_add(d, s[:, :, 0:w0 // 2],
                                     s[:, :, w0 // 2:w0])
            a4 = tr[:, 7168:7680].rearrange("p (a c) -> p a c", c=2)
            nc.vector.tensor_add(c1[:].unsqueeze(2), a4[:, :, 0:1],
                                 a4[:, :, 1:2])
            nc.vector.tensor_mul(c1[:], c1[:], wt_slice)
            nc.vector.tensor_reduce(
                ob_slice, c1[:].rearrange("p (a k) -> p a k", k=4),
                axis=X, op=ADD)

        with (
            tc.tile_pool(name="rtp", bufs=2) as rtp,
            tc.tile_pool(name="idxp", bufs=2) as idxp,
            tc.tile_pool(name="wtp", bufs=2) as wtp,
            tc.tile_pool(name="gat", bufs=2) as gat,
            tc.tile_pool(name="trp", bufs=1) as trp,
            tc.tile_pool(name="rsp", bufs=1) as rsp,
            tc.tile_pool(name="c1p", bufs=4) as c1p,
            tc.tile_pool(name="obp", bufs=2) as obp,
            tc.tile_pool(name="sop", bufs=1) as sop,
        ):
            gq = 0
            with nc.allow_low_precision(reason="f16 channel-dot accum"):
                for sc in range(NSC):
                    rt = rtp.tile([128, 2048], f16)
                    nc.sync.dma_start(rt[:], rr_d.ap()[sc])
                    idxt = idxp.tile([128, DB * 256], i16)
                    nc.sync.dma_start(idxt[:], idx_d.ap()[sc])
                    wt = wtp.tile([128, D * 64], f16)
                    nc.sync.dma_start(wt[:], wtt_d.ap()[sc])
                    obt = obp.tile([128, D * 16], f32)
                    rt_b = rt[:].unsqueeze(1).broadcast_to([128, DG, 2048])
                    for db in range(DB):
                        g = gat.tile([128, 8192], f16)
                        for h in range(DG):
                            gi = nc.gpsimd.dma_gather(
                                g[:].rearrange("p (s v) -> p s v", s=32)
                                [:, h * 8:(h + 1) * 8, :],
                                tbl_d.ap(),
                                idxt[:, db * 256 + h * 64:
                                     db * 256 + (h + 1) * 64],
                                num_idxs=1024, num_idxs_reg=1024,
                                elem_size=256, queue_num=_gather_q())
                            gather_names.append(
                                getattr(gi, 'name', None) or gi.ins.name)
                            gq += 1
                        nc.vector.tensor_mul(
                            g[:].rearrange("p (a v) -> p a v", a=DG), g[:]
                            .rearrange("p (a v) -> p a v", a=DG), rt_b)
                        tr = trp.tile([128, 7680], f16)
                        c1 = c1p.tile([128, 256], f16)
                        tree_reduce(g, tr, c1,
                                    wt[:, db * 256:(db + 1) * 256],
                                    obt[:, db * 64:(db + 1) * 64])
                    nc.sync.dma_start(outg_d.ap()[sc], obt[:])

                sit = sop.tile([128, SBLK * 256], i16, name="sit", tag="sit")
                nc.sync.dma_start(sit[:], sidx_d.ap())
                wst = sop.tile([128, NSOLO_CH * 64], f16, name="wst", tag="wst")
                nc.sync.dma_start(wst[:], wsolo_d.ap())
                ost = sop.tile([128, NSOLO_CH * 16], f32, name="ost", tag="ost")
                for blk in range(SBLK):
                    gs = gat.tile([128, 8192], f16, name="gs")
                    for h in range(DG):
                        gi = nc.gpsimd.dma_gather(
                            gs[:].rearrange("p (s v) -> p s v", s=32)
                            [:, h * 8:(h + 1) * 8, :],
                            tbl_d.ap(),
                            sit[:, blk * 256 + h * 64:
                                blk * 256 + (h + 1) * 64],
                            num_idxs=1024, num_idxs_reg=1024,
                            elem_size=256, queue_num=_gather_q())
                        gather_names.append(
                            getattr(gi, 'name', None) or gi.ins.name)
                        gq += 1
                    gsr = rsp.tile([128, 8192], f16, name="gsr")
                    nc.sync.dma_start(gsr[:], rsolo_d.ap()[blk])
                    nc.vector.tensor_mul(gs[:], gs[:], gsr[:])
                    trs = trp.tile([128, 7680], f16)
                    c1s = c1p.tile([128, 256], f16)
                    tree_reduce(gs, trs, c1s,
                                wst[:, blk * 256:(blk + 1) * 256],
                                ost[:, blk * 64:(blk + 1) * 64])
                nc.sync.dma_start(outs_d.ap(), ost[:])

    nc.compile()
    nc._gather_names = gather_names
    return nc


def _gather_lanes(nc):
    """name -> DMASW lane for every gather, from the scheduled procs."""
    lanes = {}
    for fn in nc.m.functions:
        for blk in fn.blocks:
            for inst in blk.instructions:
                if type(inst).__name__.startswith("InstDMAGather"):
                    lanes[inst.name] = inst.bass_scheduled_proc - 11
    return lanes


def _get_program():
    if "nc" in _PROGRAM_CACHE:
        return _PROGRAM_CACHE["nc"]
    # Two-pass build: SWDGE completion-sem lanes (DMASW0-7) are assigned by
    # the scheduler in module order; each lane must only ever be used from
    # one SWDGE queue. Pass 1 discovers the lane of every gather, pass 2
    # re-traces with queue = lane % 4 (consistent by construction).
    import collections
    import sys
    nc = _build_program()
    for it in range(3):
        lanes = _gather_lanes(nc)
        qmap = [lanes[n] % 4 for n in nc._gather_names]
        nc2 = _build_program(qmap)
        lanes2 = _gather_lanes(nc2)
        ok = all(lanes2[n] % 4 == qmap[i]
                 for i, n in enumerate(nc2._gather_names))
        nc = nc2
        print(f"[kernel] queue pass {it}: ok={ok} "
              f"dist={collections.Counter(qmap)}", file=sys.stderr)
        if ok:
            break
    else:
        print("[kernel] queue passes failed; all queue 0", file=sys.stderr)
        nc = _build_program()  # all queue 0: always safe
    _PROGRAM_CACHE["nc"] = nc
    return _PROGRAM_CACHE["nc"]


def _run(inputs, trace=False):
    from concourse.bass_utils import run_bass_kernel_spmd

    features = np.asarray(inputs["features"], dtype=np.float32)
    proj_matrices = np.asarray(inputs["proj_matrices"], dtype=np.float32)
    depth_values = np.asarray(inputs["depth_values"], dtype=np.float32)
    view_weights = np.asarray(inputs["view_weights"], dtype=np.float32)

    cores = _host_fields(features, proj_matrices, depth_values, view_weights)
    in_maps = _pack_core_inputs(features, cores)
    scatters = [m.pop("_scatter") for m in in_maps]
    nc = _get_program()
    res = run_bass_kernel_spmd(nc, in_maps, core_ids=list(range(NCORES)),
                               trace=trace)

    out = np.empty((B, 1, D, H, W), dtype=np.float32)
    for b in range(B):
        vol = np.zeros((D, HW), dtype=np.float32)
        wsum = np.full((HW,), 1e-5, dtype=np.float32)
        for v in range(1, V):
            ci = b * 4 + (v - 1)
            # outg [NSC,128,D*16]: d = db*4+dd; pixel = sc*2048+slab*256+2p+px
            og = (res.results[ci]["outg"].reshape(NSC, 128, DB, DG, 8, 2)
                  .transpose(2, 3, 0, 4, 1, 5).reshape(D, HW))
            osv = (res.results[ci]["outs"].reshape(128, SBLK, DG, 8, 2)
                   .transpose(1, 2, 0, 3, 4).reshape(NSOLO_CH, 128, 8, 2))
            dd, bpix = scatters[ci]
            u = np.arange(len(dd))
            og[dd, bpix] = osv[u // 1024, (u % 1024) % 128, (u % 1024) // 128, 1]
            vol = vol + og
            wsum = wsum + view_weights[b, v - 1].reshape(HW)
        out[b, 0] = (vol / wsum[None, :]).reshape(D, H, W)
    return out, res


def kernel(**inputs) -> np.ndarray:
    out, _ = _run(inputs, trace=False)
    return out
